# revision 11
# baseline (speedup 1.0000x reference)
"""HBV hydrological model (nn_HBVMulTDET_WaterLoss) as a Bass/Tile kernel on
8 Trainium2 NeuronCores.

Strategy: pure data parallelism over the 4000 grid cells (500 cells/core).
Per-core layout: partition p in [0,125) holds 4 cells x 4 components = 16
state lanes in the free dim. The T=365 recurrence is chain-latency bound
(per step: 4 DVE->ACT->DVE round trips ~750ns each plus ~10 dependent DVE
links ~210ns each), while the DVE can issue an op every ~85ns. So the 4
cells per partition are run as FOUR LOCKSTEP COHORTS of width 4: every
per-step op is emitted 4x (once per cohort, op-major), which widens each
cohort's dependency gaps by 4 issue slots and hides the serial chain
latency behind the other cohorts' instruction streams. Instruction issue
cost is width-independent at this size, so 4x narrower ops cost the same
and the step quad runs at the DVE issue-rate bound instead of the chain
bound.

All state-free derived quantities are precomputed on the host and DMAd in
ONE consolidated stream per time chunk. Algebra (verified bit-identical
to the reference recurrence in f32):
  - snow melt/refreeze collapsed into one signed flux
        X = max(min(E, SP+SNOW), -MW),  E = melt_cap - refreeze_cap
    with meltwater carried negated (NMW); tosoil = NMWn - NMW2
    (bit-identical to relu(-CWH*SP' - NMW2), one op less);
    [SPn|NMW2] produced by ONE 8-lane tensor op (X broadcast over pair)
  - soil pow() via exp/ln with host-folded log constants:
    x1 = exp(BETA*ln(SM) - BLF), x2 = exp(BETAET*ln(SM1) + LNPB)
  - ET/SM update collapsed via SM3 = max(SMc - x2, max(SMc - PET, NZ))
  - response: rech+exc == SMa-SMc, (1-K) folding with negated states,
    [NSLZn|NSUZn] produced by ONE 8-lane mult against the adjacent
    [K2Cn|K1Cn] pair of the input stream, and Q0+Q1+Q2 accumulated in
    one strided-view tensor_reduce per cohort
Engines: DVE carries the snow+soil chains and all min/max (the Pool ISA
has no tensor min/max); Pool carries the off-critical-path response
section and broadcast subtracts; ACT carries Ln/Exp/Relu only, all
resolved into the single natural_log_exp_and_others table set so the
scalar engine never reloads its activation tables. Gamma unit-hydrograph
weights are computed on host; the routing convolution runs on device.
"""
import math
import numpy as np

T_FULL = 365
NGRID = 4000
NCORES = 8
NSH = NGRID // NCORES      # 500 cells per core
PPART = 125                # partitions used
CL = 4                     # cells per partition = cohorts
M = 4                      # nmul components
NC = 4                     # lockstep cohorts (one per cell slot)
W = 4                      # lanes per cohort op (M components of one cell)
LENF = 15
NZ = 1e-5
TC = 32                    # time-chunk length
NST = 16                   # number of packed per-step streams

# stream order inside the packed dd tensor; K2Cn/K1Cn are adjacent and
# last so [NSLZn|NSUZn] = [K2Cn|K1Cn] * [SLZ2|SUZ3] is one 8-lane op
DD = ["SNOW", "E", "RAIN", "CWHn", "BETA", "BLF", "FC", "FCinv", "BETAET",
      "LNPB", "C", "PERC", "NUZL", "K0", "K2Cn", "K1Cn"]
DJ = {n: j for j, n in enumerate(DD)}

_TABLES_PATCHED = False


def _patch_act_tables():
    """Strip the functions of natural_log_exp_and_others from every other
    activation table set before the act-table-load CFG pass runs, so all
    activations resolve to that single set and the scalar engine loads its
    tables exactly once."""
    global _TABLES_PATCHED
    if _TABLES_PATCHED:
        return
    import concourse.bacc as bacc
    from concourse import hw_specs

    _orig = hw_specs.get_activation_tables
    target = "natural_log_exp_and_others"

    def _combined_only(arch):
        tables = _orig(arch)
        if target in tables:
            keep = tables[target]
            for name in list(tables):
                if name != target:
                    tables[name] = tables[name] - keep
        return tables

    bacc.get_activation_tables = _combined_only
    _TABLES_PATCHED = True


def build_program(T=T_FULL, tc_len=TC):
    _patch_act_tables()
    import concourse.bass as bass
    import concourse.bacc as bacc
    import concourse.mybir as mybir
    import concourse.tile as tile

    F32 = mybir.dt.float32
    op = mybir.AluOpType
    AF = mybir.ActivationFunctionType

    nc = bacc.Bacc("TRN2")
    dd = nc.declare_dram_parameter("dd", [PPART, T, NST, CL * M], F32,
                                   isOutput=False)
    pet = nc.declare_dram_parameter("pet", [PPART, T, CL], F32, isOutput=False)
    uh = nc.declare_dram_parameter("uh", [PPART, LENF * CL], F32, isOutput=False)
    qr = nc.declare_dram_parameter("qr", [PPART, T, CL], F32, isOutput=True)

    chunks = [(t0, min(tc_len, T - t0)) for t0 in range(0, T, tc_len)]
    R = range(NC)

    with tile.TileContext(nc) as tctx:
        with (
            tctx.tile_pool(name="blk", bufs=2) as blk_pool,
            tctx.tile_pool(name="st", bufs=6) as st_pool,
            tctx.tile_pool(name="per", bufs=1) as per_pool,
        ):
            V = nc.vector
            G = nc.gpsimd
            A = nc.scalar
            S = nc.sync

            def tt(eng, out, a, b, o):
                eng.tensor_tensor(out, a, b, o)

            Qfull = per_pool.tile([PPART, (LENF - 1 + T) * CL], F32)
            uh_t = per_pool.tile([PPART, LENF * CL], F32)
            S.dma_start(uh_t[:], uh[:])
            G.memset(Qfull[:, : (LENF - 1) * CL], 0.0)

            # ---- per-cohort state bootstrap ----
            SM = []
            TM_prev = []
            TSP_cur = []
            pc = []
            for c in R:
                s = st_pool.tile([PPART, W], F32, tag=f"SM{c}")
                G.memset(s[:], 0.001)
                SM.append(s)
                tm = st_pool.tile([PPART, 2 * W], F32, tag=f"TM{c}")
                G.memset(tm[:, 0:W], 0.001)      # SP0
                TM_prev.append(tm)
                tp = st_pool.tile([PPART, 2 * W], F32, tag=f"TSP{c}")
                G.memset(tp[:, W : 2 * W], -0.001)  # NMW0
                TSP_cur.append(tp)
                # comb: 8 slots of W lanes; lane = g*2W + x*W:
                #  g0x0 SUZ2 | g1x0 SLZ2, g1x1 SUZ3 | g2x0 NSLZn | g3x0 NSUZn
                cb = st_pool.tile([PPART, 8 * W], F32, tag=f"comb{c}")
                G.memset(cb[:, 4 * W : 5 * W], -0.001)   # NSLZ
                G.memset(cb[:, 6 * W : 7 * W], -0.001)   # NSUZ
                pc.append(cb)

            def nt(tag, c, w=W):
                return st_pool.tile([PPART, w], F32, tag=f"{tag}{c}", name=tag)

            def emit_dma(ci):
                t0, tcn = chunks[ci]
                dt_ = blk_pool.tile([PPART, tc_len * NST * 16], F32,
                                    tag="dd", name=f"dd_{t0}")
                S.dma_start(
                    dt_[:, : tcn * NST * 16].rearrange(
                        "p (t j f) -> p t j f", j=NST, f=16),
                    dd[:, t0 : t0 + tcn, :, :],
                )
                pt = blk_pool.tile([PPART, tc_len * CL], F32, tag="PET",
                                   name=f"PET_{t0}")
                S.dma_start(
                    pt[:, : tcn * CL].rearrange("p (t c) -> p t c", c=CL),
                    pet[:, t0 : t0 + tcn, :],
                )
                return {"t0": t0, "tcn": tcn, "dt": dt_, "pt": pt}

            cur = emit_dma(0)
            pendQ = [None] * NC
            pendR = [None] * NC

            def emit_pendR():
                """Deferred q-dependent response tail of the previous step,
                emitted inside the next step's snow window (Pool engine)."""
                for c in R:
                    p = pendR[c]
                    if p is None:
                        return
                    cb = p["comb"]
                    Q0 = nt("Q0", c)
                    tt(G, Q0[:], p["K0"], p["q"][:], op.mult)
                    tt(G, cb[:, 3 * W : 4 * W], cb[:, 0:W], Q0[:],
                       op.subtract)           # SUZ3
                for c in R:
                    p = pendR[c]
                    cb = p["comb"]
                    # [NSLZn|NSUZn] = [K2Cn|K1Cn] * [SLZ2|SUZ3] (one op)
                    tt(G,
                       cb[:, 4 * W : 8 * W].rearrange(
                           "p (g f) -> p g f", g=2)[:, :, 0:W],
                       p["K1K2"],
                       cb[:, 2 * W : 4 * W].rearrange("p (g f) -> p g f", g=2),
                       op.mult)

            def emit_pendQ():
                for c in R:
                    p = pendQ[c]
                    if p is None:
                        return
                    # Q0+Q1+Q2 = sum over {g, m} of the x=0 comb slots
                    V.tensor_reduce(
                        Qfull[:, (LENF - 1 + p["t"]) * CL + c :
                              (LENF - 1 + p["t"]) * CL + c + 1],
                        p["comb"][:].rearrange("p (g x m) -> p x g m",
                                               g=4, x=2, m=M)[:, 0],
                        axis=mybir.AxisListType.XY,
                        op=op.add,
                    )

            for ci in range(len(chunks)):
                nxt = emit_dma(ci + 1) if ci + 1 < len(chunks) else None
                t0, tcn = cur["t0"], cur["tcn"]
                dt_ = cur["dt"]
                pt_ = cur["pt"]

                for ti in range(tcn):
                    t = t0 + ti

                    def cs(name, c):
                        base = ti * NST * 16 + DJ[name] * 16 + c * W
                        return dt_[:, base : base + W]

                    # ---- kick off the soil ACT chain ----
                    lnSM = [nt("lnSM", c) for c in R]
                    for c in R:
                        A.activation(lnSM[c][:], SM[c][:], AF.Ln)

                    # ---- snow (DVE; fills the lnSM window) ----
                    for c in R:
                        tt(V, TSP_cur[c][:, 0:W], TM_prev[c][:, 0:W],
                           cs("SNOW", c), op.add)          # SPa
                    mn = [nt("mn", c) for c in R]
                    for c in R:
                        tt(V, mn[c][:], cs("E", c), TSP_cur[c][:, 0:W], op.min)
                    X = [nt("X", c) for c in R]
                    for c in R:
                        tt(V, X[c][:], mn[c][:], TSP_cur[c][:, W : 2 * W],
                           op.max)
                    TM = [nt("TMn", c, 2 * W) for c in R]
                    for c in R:
                        tt(V,
                           TM[c][:].rearrange("p (g f) -> p g f", g=2),
                           TSP_cur[c][:].rearrange("p (g f) -> p g f", g=2),
                           X[c][:].unsqueeze(1).to_broadcast((PPART, 2, W)),
                           op.subtract)                    # [SPn | NMW2]
                    NW = [nt("NW", c) for c in R]
                    for c in R:
                        tt(V, NW[c][:], cs("CWHn", c), TM[c][:, 0:W], op.mult)
                    TSP_next = [st_pool.tile([PPART, 2 * W], F32,
                                             tag=f"TSP{c}", name="TSP")
                                for c in R]
                    for c in R:
                        tt(V, TSP_next[c][:, W : 2 * W], TM[c][:, W : 2 * W],
                           NW[c][:], op.max)               # NMWn
                    tosp = [nt("tosp", c) for c in R]
                    for c in R:
                        tt(V, tosp[c][:], TSP_next[c][:, W : 2 * W],
                           TM[c][:, W : 2 * W], op.subtract)
                    wi = [nt("wi", c) for c in R]
                    for c in R:
                        tt(V, wi[c][:], cs("RAIN", c), tosp[c][:], op.add)
                    TM_prev = TM
                    TSP_cur = TSP_next

                    # previous step's deferred response tail (Pool)
                    emit_pendR()
                    CnSLZ = [nt("CnSLZ", c) for c in R]
                    for c in R:
                        tt(G, CnSLZ[c][:], cs("C", c), pc[c][:, 4 * W : 5 * W],
                           op.mult)

                    # ---- soil on-path ----
                    SMa = [nt("SMa", c) for c in R]
                    for c in R:
                        tt(V, SMa[c][:], SM[c][:], wi[c][:], op.add)
                    v = [nt("v", c) for c in R]
                    for c in R:
                        tt(V, v[c][:], lnSM[c][:], cs("BETA", c), op.mult)
                    u = [nt("u", c) for c in R]
                    for c in R:
                        tt(V, u[c][:], v[c][:], cs("BLF", c), op.subtract)
                    x1 = [nt("x1", c) for c in R]
                    for c in R:
                        A.activation(x1[c][:], u[c][:], AF.Exp)

                    # x1 window: previous step's Q reduces
                    emit_pendQ()

                    rech = [nt("rech", c) for c in R]
                    for c in R:
                        V.scalar_tensor_tensor(rech[c][:], x1[c][:], 1.0,
                                               wi[c][:], op.min, op.mult)
                    SM1 = [nt("SM1", c) for c in R]
                    for c in R:
                        tt(V, SM1[c][:], SMa[c][:], rech[c][:], op.subtract)
                    ln2 = [nt("ln2", c) for c in R]
                    for c in R:
                        A.activation(ln2[c][:], SM1[c][:], AF.Ln)

                    # ln2 window: SMc, ET floor and the response head
                    SMc = [nt("SMc", c) for c in R]
                    for c in R:
                        tt(V, SMc[c][:], SM1[c][:], cs("FC", c), op.min)
                    SMcP = [nt("SMcP", c) for c in R]
                    for c in R:
                        petb = pt_[:, ti * CL + c : ti * CL + c + 1]\
                            .to_broadcast((PPART, W))
                        tt(G, SMcP[c][:], SMc[c][:], petb, op.subtract)
                    SMcP2 = [nt("SMcP2", c) for c in R]
                    for c in R:
                        V.tensor_scalar_max(SMcP2[c][:], SMcP[c][:], NZ)
                    SUZ1a = [nt("SUZ1a", c) for c in R]
                    for c in R:
                        tt(G, SUZ1a[c][:], SMa[c][:], pc[c][:, 6 * W : 7 * W],
                           op.subtract)
                    SUZ1 = [nt("SUZ1", c) for c in R]
                    for c in R:
                        tt(G, SUZ1[c][:], SUZ1a[c][:], SMc[c][:], op.subtract)
                    PERCa = [nt("PERCa", c) for c in R]
                    for c in R:
                        tt(V, PERCa[c][:], SUZ1[c][:], cs("PERC", c), op.min)
                    comb = [st_pool.tile([PPART, 8 * W], F32, tag=f"comb{c}",
                                         name="comb") for c in R]
                    for c in R:
                        tt(G, comb[c][:, 0:W], SUZ1[c][:], PERCa[c][:],
                           op.subtract)                    # SUZ2
                    t5 = [nt("t5", c) for c in R]
                    for c in R:
                        tt(G, t5[c][:], comb[c][:, 0:W], cs("NUZL", c), op.add)
                    q = [nt("q", c) for c in R]
                    for c in R:
                        A.activation(q[c][:], t5[c][:], AF.Relu)

                    # ---- on-path: w2 = BETAET*ln2 + LNPB ----
                    v2 = [nt("v2", c) for c in R]
                    for c in R:
                        tt(V, v2[c][:], ln2[c][:], cs("BETAET", c), op.mult)
                    w2 = [nt("w2", c) for c in R]
                    for c in R:
                        tt(V, w2[c][:], v2[c][:], cs("LNPB", c), op.add)
                    x2 = [nt("x2", c) for c in R]
                    for c in R:
                        A.activation(x2[c][:], w2[c][:], AF.Exp)

                    # ---- on-path tail: SM3, capillary, SM ----
                    tq = [nt("tq", c) for c in R]
                    for c in R:
                        V.scalar_tensor_tensor(tq[c][:], x2[c][:], -1.0,
                                               SMc[c][:], op.mult, op.add)
                    SM3 = [nt("SM3", c) for c in R]
                    for c in R:
                        tt(V, SM3[c][:], tq[c][:], SMcP2[c][:], op.max)
                    g = [nt("g", c) for c in R]
                    for c in R:
                        tt(V, g[c][:], SM3[c][:], cs("FCinv", c), op.mult)
                    rln = [nt("rln", c) for c in R]
                    for c in R:
                        V.tensor_scalar(rln[c][:], g[c][:], 1.0, 1.0,
                                        op.min, op.subtract)
                    cap = [nt("cap", c) for c in R]
                    for c in R:
                        tt(V, cap[c][:], CnSLZ[c][:], rln[c][:], op.mult)
                    SMn = [nt("SM", c) for c in R]
                    for c in R:
                        tt(V, SMn[c][:], SM3[c][:], cap[c][:], op.add)
                    SM = SMn

                    # ---- response tail ----
                    sl_n = [nt("sl_n", c) for c in R]
                    for c in R:
                        tt(G, sl_n[c][:], pc[c][:, 4 * W : 5 * W], cap[c][:],
                           op.add)
                    NSLZ1 = [nt("NSLZ1", c) for c in R]
                    for c in R:
                        V.tensor_scalar_min(NSLZ1[c][:], sl_n[c][:], -NZ)
                    for c in R:
                        tt(G, comb[c][:, 2 * W : 3 * W], PERCa[c][:],
                           NSLZ1[c][:], op.subtract)       # SLZ2

                    kkbase = ti * NST * 16 + DJ["K2Cn"] * 16
                    for c in R:
                        K1K2 = dt_[:, kkbase : kkbase + 32].rearrange(
                            "p (g f) -> p g f", g=2)[:, :, c * W : (c + 1) * W]
                        pendR[c] = {"comb": comb[c], "q": q[c],
                                    "K0": cs("K0", c), "K1K2": K1K2}
                        pendQ[c] = {"t": t, "comb": comb[c]}
                    pc = comb

                if nxt is not None:
                    cur = nxt

            emit_pendR()
            emit_pendQ()

            # ---- gamma-UH routing (DVE, bulk over all cohorts) ----
            Qr = per_pool.tile([PPART, T * CL], F32)
            prod = per_pool.tile([PPART, T * CL], F32)

            def qr4(ap_):
                return ap_.rearrange("p (t c) -> p t c", c=CL)

            for k in range(LENF):
                sh = Qfull[:, (LENF - 1 - k) * CL : (LENF - 1 - k + T) * CL]
                uhk = (
                    uh_t[:, k * CL : (k + 1) * CL]
                    .unsqueeze(1)
                    .to_broadcast((PPART, T, CL))
                )
                if k == 0:
                    tt(V, qr4(Qr[:]), uhk, qr4(sh), op.mult)
                else:
                    tt(V, qr4(prod[:]), uhk, qr4(sh), op.mult)
                    tt(V, qr4(Qr[:]), qr4(Qr[:]), qr4(prod[:]), op.add)

            S.dma_start(qr[:, :, :], Qr[:].rearrange("p (t c) -> p t c", c=CL))

    return nc


# ---------------- host-side packing ----------------

def _derived_full(x_hydro_model, params_raw):
    """All state-free per-step tensors, float32, shapes [T, N, M] (per-cell
    quantities broadcast over M)."""
    f32 = np.float32
    T, N, _ = x_hydro_model.shape
    raw = np.ascontiguousarray(params_raw[:, :, :14, :], dtype=f32)
    x = np.ascontiguousarray(x_hydro_model, dtype=f32)
    P = x[:, :, 0:1]
    Ta = x[:, :, 1:2]
    PET = x[:, :, 2:3]

    BETA = f32(5.0) * raw[:, :, 0] + f32(1.0)
    FC = f32(950.0) * raw[:, :, 1] + f32(50.0)
    K0 = f32(0.85) * raw[:, :, 2] + f32(0.05)
    K1Cn = f32(0.49) * raw[:, :, 3] - f32(0.99)
    K2Cn = f32(0.199) * raw[:, :, 4] - f32(0.999)
    LP = f32(0.8) * raw[:, :, 5] + f32(0.2)
    PERC = f32(10.0) * raw[:, :, 6]
    NUZL = f32(-100.0) * raw[:, :, 7]
    TTn = f32(-5.0) * raw[:, :, 8] + f32(2.5)
    CFMX = f32(9.5) * raw[:, :, 9] + f32(0.5)
    CWHn = f32(-0.2) * raw[:, :, 11]
    BETAET = f32(4.7) * raw[:, :, 12] + f32(0.3)
    C = raw[:, :, 13]

    Tdiff = (Ta + TTn).astype(f32)
    m1 = (CFMX * Tdiff).astype(f32)
    rn = np.maximum(-m1, 0).astype(f32)
    Rc0 = ((f32(0.1) * raw[:, :, 10]).astype(f32) * rn).astype(f32)
    Gc0 = np.maximum(m1, 0).astype(f32)
    E = (Gc0 - Rc0).astype(f32)
    mask = (Tdiff >= 0).astype(f32)
    RAIN = (mask * P).astype(f32)
    SNOW = (P - RAIN).astype(f32)
    lnFC = np.log(FC).astype(f32)
    FCinv = np.exp(-lnFC).astype(f32)
    BLF = (BETA * lnFC).astype(f32)
    LPFC = (LP * FC).astype(f32)
    lnLPFC = np.log(LPFC).astype(f32)
    BL2 = (BETAET * lnLPFC).astype(f32)
    lnPET = np.log(np.maximum(PET, f32(1e-30))).astype(f32)
    LNPB = (lnPET - BL2).astype(f32)

    return {
        "E": E, "SNOW": SNOW, "RAIN": RAIN, "CWHn": CWHn, "BETA": BETA,
        "BLF": BLF, "FC": FC, "FCinv": FCinv, "BETAET": BETAET, "LNPB": LNPB,
        "C": C, "PERC": PERC, "NUZL": NUZL, "K0": K0, "K1Cn": K1Cn,
        "K2Cn": K2Cn,
    }


def pack_inputs(x_hydro_model, params_raw, conv_params_hydro):
    T = x_hydro_model.shape[0]
    f32 = np.float32
    der = _derived_full(x_hydro_model, params_raw)
    # [T, N, M] -> per core [PPART, T, NST, CL*M]
    dd_full = np.stack([der[n] for n in DD], axis=0)  # [nd, T, N, M]
    nd = dd_full.shape[0]
    dd_c = (dd_full.reshape(nd, T, NCORES, PPART, CL * M)
            .transpose(2, 3, 1, 0, 4))           # [cores, P, T, nd, 16]

    PET = np.ascontiguousarray(x_hydro_model[:, :, 2], dtype=f32)  # [T, N]
    pet_c = PET.reshape(T, NCORES, PPART, CL).transpose(1, 2, 0, 3)

    conv = np.asarray(conv_params_hydro, dtype=np.float64)
    a = conv[:, 0] * 2.9
    b = conv[:, 1] * 6.5
    aa = np.maximum(a, 0) + 0.1
    theta = np.maximum(b, 0) + 0.5
    tgrid = np.arange(0.5, float(LENF), dtype=np.float64)[:, None]
    lg = np.array([math.lgamma(v) for v in aa])
    w = np.exp(-lg) / theta ** aa * tgrid ** (aa - 1.0) * np.exp(-tgrid / theta)
    w = w / w.sum(0)
    UH = (w * (1.0 / M)).astype(f32)  # [LENF, NGRID], mean-over-M folded in
    uh_c = UH.reshape(LENF, NCORES, PPART, CL).transpose(1, 2, 0, 3)

    in_maps = []
    for i in range(NCORES):
        in_maps.append({
            "dd": np.ascontiguousarray(dd_c[i]),
            "pet": np.ascontiguousarray(pet_c[i]),
            "uh": np.ascontiguousarray(uh_c[i]).reshape(PPART, LENF * CL),
        })
    return in_maps


def unpack_outputs(results, T):
    out = np.empty((T, NGRID), np.float32)
    for i in range(NCORES):
        q = results[i]["qr"].reshape(PPART, T, CL)
        out[:, i * NSH : (i + 1) * NSH] = q.transpose(1, 0, 2).reshape(T, NSH)
    return out


_PROG_CACHE = {}


def kernel(x_hydro_model, params_raw, conv_params_hydro):
    from concourse.bass_utils import run_bass_kernel_spmd

    T = x_hydro_model.shape[0]
    key = T
    if key not in _PROG_CACHE:
        _PROG_CACHE[key] = build_program(T=T)
    nc = _PROG_CACHE[key]
    if not nc.is_finalized():
        nc.finalize()
    in_maps = pack_inputs(x_hydro_model, params_raw, conv_params_hydro)
    res = run_bass_kernel_spmd(nc, in_maps, list(range(NCORES)))
    return unpack_outputs(res.results, T)


# revision 16
# speedup vs baseline: 1.6495x; 1.6495x over previous
"""HBV hydrological model (nn_HBVMulTDET_WaterLoss) as a Bass/Tile kernel on
8 Trainium2 NeuronCores.

Strategy: pure data parallelism over the 4000 grid cells (500 cells/core).
Per-core layout: partition p in [0,125) holds 4 cells x 4 components = 16
state lanes in the free dim. All state-free derived quantities are
precomputed on the host, so the device program is a pure steady-state
recurrence stream: the T=365 step loop fully unrolled.

The step time is bound by the serial soil-moisture chain (four DVE->ACT
->DVE round trips ~750ns each plus the dependent DVE links ~210ns each),
so the program is software-pipelined depth 2 around it: step t's emission
carries ONLY the soil chain of step t; the snow chain of step t+1 and the
q-dependent response tail + Q reduce of step t-1 are placed inside step
t's ACT wait windows; the off-critical response section runs on the Pool
engine (whose ISA has no tensor min/max, so all clamps sit on the DVE).
The ACT queue order is lnSM, x1, ln2, x2, then q, so a late response head
can never head-of-line-block the chain activations.

Algebraic restructuring vs the reference:
  - snow melt/refreeze collapsed into one signed flux
        X = max(min(E, SP+SNOW), -MW),  E = melt_cap - refreeze_cap
    meltwater carried negated (NMW); tosoil = NMWn - NMW2 (bit-identical
    to relu(-CWH*SP' - NMW2)); [SPn|NMW2] is ONE 32-lane op (X broadcast)
  - soil pow() via exp/ln: x1 = exp(BETA*ln(SM) - BLF),
    x2 = PET*evap = exp(BETAET*ln(SM1) + lnPET - BETAET*ln(LP*FC))
  - ET/SM update collapsed via SM3 = max(SMc - x2, max(SMc - PET, NZ))
  - capillary in W-form: SM' = SM3*(1 - C*SLZ/FC) + C*SLZ, with the two
    coefficients computed off-path on Pool from the previous SLZ, which
    shortens the on-chain tail from 4 links to 2 (costs ~7e-3 extra
    rounding error from the large-term cancellation, measured 1.41e-2
    total vs the 2e-2 gate)
  - response: rech+exc == SMa-SMc, (1-K) folding with negated states,
    [NSLZn|NSUZn] in ONE 32-lane mult against the adjacent [K2Cn|K1Cn]
    input pair, Q0+Q1+Q2 in one strided-view tensor_reduce per step
All activations resolve into the single natural_log_exp_and_others table
set so the scalar engine never reloads its tables. Gamma unit-hydrograph
weights are computed on host; the routing convolution runs on device.
"""
import math
import numpy as np

T_FULL = 365
NGRID = 4000
NCORES = 8
NSH = NGRID // NCORES      # 500 cells per core
PPART = 125                # partitions used
CL = 4                     # cells per partition
M = 4                      # nmul components
LENF = 15
NZ = 1e-5
TC = 32                    # time-chunk length
NST = 16                   # number of packed per-step streams
WFORM = True               # 2-link capillary tail (False: 4-link exact)

# stream order inside the packed dd tensor; K2Cn/K1Cn are adjacent and
# last so [NSLZn|NSUZn] = [K2Cn|K1Cn] * [SLZ2|SUZ3] is one 32-lane op
DD = ["SNOW", "E", "RAIN", "CWHn", "BETA", "BLF", "FC", "FCinv", "BETAET",
      "LNPB", "C", "PERC", "NUZL", "K0", "K2Cn", "K1Cn"]
DJ = {n: j for j, n in enumerate(DD)}

_TABLES_PATCHED = False


def _patch_act_tables():
    """Strip the functions of natural_log_exp_and_others from every other
    activation table set before the act-table-load CFG pass runs, so all
    activations resolve to that single set and the scalar engine loads its
    tables exactly once."""
    global _TABLES_PATCHED
    if _TABLES_PATCHED:
        return
    import concourse.bacc as bacc
    from concourse import hw_specs

    _orig = hw_specs.get_activation_tables
    target = "natural_log_exp_and_others"

    def _combined_only(arch):
        tables = _orig(arch)
        if target in tables:
            keep = tables[target]
            for name in list(tables):
                if name != target:
                    tables[name] = tables[name] - keep
        return tables

    bacc.get_activation_tables = _combined_only
    _TABLES_PATCHED = True


def build_program(T=T_FULL, tc_len=TC):
    _patch_act_tables()
    import concourse.bass as bass
    import concourse.bacc as bacc
    import concourse.mybir as mybir
    import concourse.tile as tile

    F32 = mybir.dt.float32
    op = mybir.AluOpType
    AF = mybir.ActivationFunctionType

    nc = bacc.Bacc("TRN2")
    dd = nc.declare_dram_parameter("dd", [PPART, T, NST, CL * M], F32,
                                   isOutput=False)
    pet = nc.declare_dram_parameter("pet", [PPART, T, CL], F32, isOutput=False)
    uh = nc.declare_dram_parameter("uh", [PPART, LENF * CL], F32, isOutput=False)
    qr = nc.declare_dram_parameter("qr", [PPART, T, CL], F32, isOutput=True)

    chunks = [(t0, min(tc_len, T - t0)) for t0 in range(0, T, tc_len)]
    n_chunks = len(chunks)

    with tile.TileContext(nc) as tctx:
        with (
            tctx.tile_pool(name="blk", bufs=2) as blk_pool,
            tctx.tile_pool(name="st", bufs=6) as st_pool,
            tctx.tile_pool(name="per", bufs=1) as per_pool,
        ):
            V = nc.vector
            G = nc.gpsimd
            A = nc.scalar
            S = nc.sync

            def tt(eng, out, a, b, o):
                eng.tensor_tensor(out, a, b, o)

            Qfull = per_pool.tile([PPART, (LENF - 1 + T) * CL], F32)
            uh_t = per_pool.tile([PPART, LENF * CL], F32)
            S.dma_start(uh_t[:], uh[:])
            G.memset(Qfull[:, : (LENF - 1) * CL], 0.0)

            # ---- state bootstrap ----
            SM = st_pool.tile([PPART, 16], F32, tag="SM")
            G.memset(SM[:], 0.001)
            TM_prev = st_pool.tile([PPART, 32], F32, tag="TM")
            G.memset(TM_prev[:, 0:16], 0.001)      # SP0
            TSP_cur = st_pool.tile([PPART, 32], F32, tag="TSP")
            G.memset(TSP_cur[:, 16:32], -0.001)    # NMW0
            # comb: 8 slots of 16 lanes; lane = g*32 + x*16:
            #  g0x0 SUZ2 | g1x0 SLZ2, g1x1 SUZ3 | g2x0 NSLZn | g3x0 NSUZn
            pc = st_pool.tile([PPART, 128], F32, tag="comb")
            G.memset(pc[:, 64:80], -0.001)    # NSLZ
            G.memset(pc[:, 96:112], -0.001)   # NSUZ

            def nt(tag):
                return st_pool.tile([PPART, 16], F32, tag=tag, name=tag)

            def emit_dma(ci):
                t0, tcn = chunks[ci]
                dt_ = blk_pool.tile([PPART, tc_len * NST * 16], F32,
                                    tag="dd", name=f"dd_{t0}")
                S.dma_start(
                    dt_[:, : tcn * NST * 16].rearrange(
                        "p (t j f) -> p t j f", j=NST, f=16),
                    dd[:, t0 : t0 + tcn, :, :],
                )
                pt = blk_pool.tile([PPART, tc_len * CL], F32, tag="PET",
                                   name=f"PET_{t0}")
                S.dma_start(
                    pt[:, : tcn * CL].rearrange("p (t c) -> p t c", c=CL),
                    pet[:, t0 : t0 + tcn, :],
                )
                petb = (
                    pt[:, : tcn * CL]
                    .rearrange("p (t c) -> p t c", c=CL)
                    .unsqueeze(3)
                    .to_broadcast((PPART, tcn, CL, M))
                )
                return {"t0": t0, "tcn": tcn, "dt": dt_, "PETb": petb}

            bufs = [emit_dma(0)]

            def cs_at(t, name):
                ci = t // tc_len
                b = bufs[ci]
                ti = t - b["t0"]
                base = ti * NST * 16 + DJ[name] * 16
                return b["dt"][:, base : base + 16]

            # ---- snow sub-chain (DVE), software-pipelined one step ahead:
            # results land in sn dicts keyed by step.
            def emit_snow_a(t):
                """SPa, mn, X, [SPn|NMW2] for step t."""
                nonlocal TSP_cur
                tt(V, TSP_cur[:, 0:16], TM_prev[:, 0:16], cs_at(t, "SNOW"),
                   op.add)                         # SPa
                mn = nt("mn")
                tt(V, mn[:], cs_at(t, "E"), TSP_cur[:, 0:16], op.min)
                X = nt("X")
                tt(V, X[:], mn[:], TSP_cur[:, 16:32], op.max)
                TM = st_pool.tile([PPART, 32], F32, tag="TMn", name="TM")
                tt(V,
                   TM[:].rearrange("p (g f) -> p g f", g=2),
                   TSP_cur[:].rearrange("p (g f) -> p g f", g=2),
                   X[:].unsqueeze(1).to_broadcast((PPART, 2, 16)),
                   op.subtract)                    # [SPn | NMW2]
                return TM

            def emit_snow_b(t, TM):
                """NW, NMWn, tosp, wi for step t; advances TM_prev/TSP_cur."""
                nonlocal TM_prev, TSP_cur
                NW = nt("NW")
                tt(V, NW[:], cs_at(t, "CWHn"), TM[:, 0:16], op.mult)
                TSP_next = st_pool.tile([PPART, 32], F32, tag="TSP",
                                        name="TSP")
                tt(V, TSP_next[:, 16:32], TM[:, 16:32], NW[:], op.max)  # NMWn
                tosp = nt("tosp")
                tt(V, tosp[:], TSP_next[:, 16:32], TM[:, 16:32], op.subtract)
                wi = nt("wi")
                tt(V, wi[:], cs_at(t, "RAIN"), tosp[:], op.add)
                TM_prev = TM
                TSP_cur = TSP_next
                return wi

            pendR = None
            pendQ = None

            def emit_pendR(p):
                """q-dependent response tail of the previous step (Pool)."""
                if p is None:
                    return
                cb = p["comb"]
                Q0 = nt("Q0")
                tt(G, Q0[:], p["K0"], p["q"][:], op.mult)
                tt(G, cb[:, 48:64], cb[:, 0:16], Q0[:], op.subtract)  # SUZ3
                # [NSLZn|NSUZn] = [K2Cn|K1Cn] * [SLZ2|SUZ3]  (one op)
                tt(G,
                   cb[:, 64:128].rearrange("p (g f) -> p g f", g=2)[:, :, 0:16],
                   p["K1K2"],
                   cb[:, 32:64].rearrange("p (g f) -> p g f", g=2),
                   op.mult)

            def emit_pendQ(p):
                if p is None:
                    return
                V.tensor_reduce(
                    Qfull[:, (LENF - 1 + p["t"]) * CL : (LENF + p["t"]) * CL],
                    p["comb"][:].rearrange("p (g x c m) -> p x c g m",
                                           g=4, x=2, m=M)[:, 0],
                    axis=mybir.AxisListType.XY,
                    op=op.add,
                )

            # ---- prologue: snow for step 0 ----
            TM0 = emit_snow_a(0)
            wi = emit_snow_b(0, TM0)

            for t in range(T):
                ci = t // tc_len
                # prefetch the next chunk a full chunk ahead
                if t % tc_len == 0 and ci + 1 < n_chunks and len(bufs) == ci + 1:
                    bufs.append(emit_dma(ci + 1))

                def cs(name):
                    return cs_at(t, name)

                # ---- chain head ----
                lnSM = nt("lnSM")
                A.activation(lnSM[:], SM[:], AF.Ln)

                # lnSM window: prev response tail (Pool) + W-prep (Pool)
                emit_pendR(pendR)
                NSLZ = pc[:, 64:80]
                NSUZ = pc[:, 96:112]
                CnSLZ = nt("CnSLZ")
                tt(G, CnSLZ[:], cs("C"), NSLZ, op.mult)
                if WFORM:
                    CF = nt("CF")
                    tt(G, CF[:], CnSLZ[:], cs("FCinv"), op.mult)

                SMa = nt("SMa")
                tt(V, SMa[:], SM[:], wi[:], op.add)
                if WFORM:
                    Wc = nt("Wc")
                    V.tensor_scalar_add(Wc[:], CF[:], 1.0)
                v = nt("v")
                tt(V, v[:], lnSM[:], cs("BETA"), op.mult)
                u = nt("u")
                tt(V, u[:], v[:], cs("BLF"), op.subtract)
                x1 = nt("x1")
                A.activation(x1[:], u[:], AF.Exp)

                # x1 window: next step's snow head + prev step's Q reduce
                TMn = emit_snow_a(t + 1) if t + 1 < T else None
                emit_pendQ(pendQ)

                rech = nt("rech")
                V.scalar_tensor_tensor(rech[:], x1[:], 1.0, wi[:],
                                       op.min, op.mult)
                SM1 = nt("SM1")
                tt(V, SM1[:], SMa[:], rech[:], op.subtract)
                ln2 = nt("ln2")
                A.activation(ln2[:], SM1[:], AF.Ln)

                # ln2 window: next step's snow tail + SMc + response head
                wi_next = emit_snow_b(t + 1, TMn) if t + 1 < T else None
                SMc = nt("SMc")
                tt(V, SMc[:], SM1[:], cs("FC"), op.min)
                SMcP = nt("SMcP")
                tt(G, SMcP[:].rearrange("p (c m) -> p c m", m=M),
                   SMc[:].rearrange("p (c m) -> p c m", m=M),
                   bufs[ci]["PETb"][:, t - bufs[ci]["t0"], :, :], op.subtract)
                SMcP2 = nt("SMcP2")
                V.tensor_scalar_max(SMcP2[:], SMcP[:], NZ)
                SUZ1a = nt("SUZ1a")
                tt(G, SUZ1a[:], SMa[:], NSUZ, op.subtract)
                SUZ1 = nt("SUZ1")
                tt(G, SUZ1[:], SUZ1a[:], SMc[:], op.subtract)
                PERCa = nt("PERCa")
                tt(V, PERCa[:], SUZ1[:], cs("PERC"), op.min)
                comb = st_pool.tile([PPART, 128], F32, tag="comb",
                                    name="comb")
                tt(G, comb[:, 0:16], SUZ1[:], PERCa[:], op.subtract)  # SUZ2
                t5 = nt("t5")
                tt(G, t5[:], comb[:, 0:16], cs("NUZL"), op.add)

                v2 = nt("v2")
                tt(V, v2[:], ln2[:], cs("BETAET"), op.mult)
                w2 = nt("w2")
                tt(V, w2[:], v2[:], cs("LNPB"), op.add)
                x2 = nt("x2")
                A.activation(x2[:], w2[:], AF.Exp)
                q = nt("q")
                A.activation(q[:], t5[:], AF.Relu)

                # ---- chain tail ----
                tq = nt("tq")
                V.scalar_tensor_tensor(tq[:], x2[:], -1.0, SMc[:],
                                       op.mult, op.add)
                SM3 = nt("SM3")
                tt(V, SM3[:], tq[:], SMcP2[:], op.max)
                if WFORM:
                    pp = nt("pp")
                    tt(V, pp[:], SM3[:], Wc[:], op.mult)
                    SMn = nt("SM")
                    tt(V, SMn[:], pp[:], CnSLZ[:], op.subtract)
                else:
                    g_ = nt("g")
                    tt(V, g_[:], SM3[:], cs("FCinv"), op.mult)
                    rln = nt("rln")
                    V.tensor_scalar(rln[:], g_[:], 1.0, 1.0, op.min,
                                    op.subtract)
                    capv = nt("capv")
                    tt(V, capv[:], CnSLZ[:], rln[:], op.mult)
                    SMn = nt("SM")
                    tt(V, SMn[:], SM3[:], capv[:], op.add)
                SM = SMn

                # ---- response tail (off-chain) ----
                cap = nt("cap")
                tt(G, cap[:], SMn[:], SM3[:], op.subtract)
                sl_n = nt("sl_n")
                tt(G, sl_n[:], NSLZ, cap[:], op.add)
                NSLZ1 = nt("NSLZ1")
                V.tensor_scalar_min(NSLZ1[:], sl_n[:], -NZ)
                tt(G, comb[:, 32:48], PERCa[:], NSLZ1[:], op.subtract)  # SLZ2

                kkbase = (t - bufs[ci]["t0"]) * NST * 16 + DJ["K2Cn"] * 16
                K1K2 = bufs[ci]["dt"][:, kkbase : kkbase + 32].rearrange(
                    "p (g f) -> p g f", g=2)
                pendR = {"comb": comb, "q": q, "K0": cs("K0"), "K1K2": K1K2}
                pendQ = {"t": t, "comb": comb}
                pc = comb
                wi = wi_next

            emit_pendR(pendR)
            emit_pendQ(pendQ)

            # ---- gamma-UH routing (DVE, bulk) ----
            Qr = per_pool.tile([PPART, T * CL], F32)
            prod = per_pool.tile([PPART, T * CL], F32)

            def qr4(ap_):
                return ap_.rearrange("p (t c) -> p t c", c=CL)

            for k in range(LENF):
                sh = Qfull[:, (LENF - 1 - k) * CL : (LENF - 1 - k + T) * CL]
                uhk = (
                    uh_t[:, k * CL : (k + 1) * CL]
                    .unsqueeze(1)
                    .to_broadcast((PPART, T, CL))
                )
                if k == 0:
                    tt(V, qr4(Qr[:]), uhk, qr4(sh), op.mult)
                else:
                    tt(V, qr4(prod[:]), uhk, qr4(sh), op.mult)
                    tt(V, qr4(Qr[:]), qr4(Qr[:]), qr4(prod[:]), op.add)

            S.dma_start(qr[:, :, :], Qr[:].rearrange("p (t c) -> p t c", c=CL))

    return nc


# ---------------- host-side packing ----------------

def _derived_full(x_hydro_model, params_raw):
    """All state-free per-step tensors, float32, shapes [T, N, M] (per-cell
    quantities broadcast over M)."""
    f32 = np.float32
    T, N, _ = x_hydro_model.shape
    raw = np.ascontiguousarray(params_raw[:, :, :14, :], dtype=f32)
    x = np.ascontiguousarray(x_hydro_model, dtype=f32)
    P = x[:, :, 0:1]
    Ta = x[:, :, 1:2]
    PET = x[:, :, 2:3]

    BETA = f32(5.0) * raw[:, :, 0] + f32(1.0)
    FC = f32(950.0) * raw[:, :, 1] + f32(50.0)
    K0 = f32(0.85) * raw[:, :, 2] + f32(0.05)
    K1Cn = f32(0.49) * raw[:, :, 3] - f32(0.99)
    K2Cn = f32(0.199) * raw[:, :, 4] - f32(0.999)
    LP = f32(0.8) * raw[:, :, 5] + f32(0.2)
    PERC = f32(10.0) * raw[:, :, 6]
    NUZL = f32(-100.0) * raw[:, :, 7]
    TTn = f32(-5.0) * raw[:, :, 8] + f32(2.5)
    CFMX = f32(9.5) * raw[:, :, 9] + f32(0.5)
    CWHn = f32(-0.2) * raw[:, :, 11]
    BETAET = f32(4.7) * raw[:, :, 12] + f32(0.3)
    C = raw[:, :, 13]

    Tdiff = (Ta + TTn).astype(f32)
    m1 = (CFMX * Tdiff).astype(f32)
    rn = np.maximum(-m1, 0).astype(f32)
    Rc0 = ((f32(0.1) * raw[:, :, 10]).astype(f32) * rn).astype(f32)
    Gc0 = np.maximum(m1, 0).astype(f32)
    E = (Gc0 - Rc0).astype(f32)
    mask = (Tdiff >= 0).astype(f32)
    RAIN = (mask * P).astype(f32)
    SNOW = (P - RAIN).astype(f32)
    lnFC = np.log(FC).astype(f32)
    FCinv = np.exp(-lnFC).astype(f32)
    BLF = (BETA * lnFC).astype(f32)
    LPFC = (LP * FC).astype(f32)
    lnLPFC = np.log(LPFC).astype(f32)
    BL2 = (BETAET * lnLPFC).astype(f32)
    lnPET = np.log(np.maximum(PET, f32(1e-30))).astype(f32)
    LNPB = (lnPET - BL2).astype(f32)

    return {
        "E": E, "SNOW": SNOW, "RAIN": RAIN, "CWHn": CWHn, "BETA": BETA,
        "BLF": BLF, "FC": FC, "FCinv": FCinv, "BETAET": BETAET, "LNPB": LNPB,
        "C": C, "PERC": PERC, "NUZL": NUZL, "K0": K0, "K1Cn": K1Cn,
        "K2Cn": K2Cn,
    }


def pack_inputs(x_hydro_model, params_raw, conv_params_hydro):
    T = x_hydro_model.shape[0]
    f32 = np.float32
    der = _derived_full(x_hydro_model, params_raw)
    # [T, N, M] -> per core [PPART, T, NST, CL*M]
    dd_full = np.stack([der[n] for n in DD], axis=0)  # [nd, T, N, M]
    nd = dd_full.shape[0]
    dd_c = (dd_full.reshape(nd, T, NCORES, PPART, CL * M)
            .transpose(2, 3, 1, 0, 4))           # [cores, P, T, nd, 16]

    PET = np.ascontiguousarray(x_hydro_model[:, :, 2], dtype=f32)  # [T, N]
    pet_c = PET.reshape(T, NCORES, PPART, CL).transpose(1, 2, 0, 3)

    conv = np.asarray(conv_params_hydro, dtype=np.float64)
    a = conv[:, 0] * 2.9
    b = conv[:, 1] * 6.5
    aa = np.maximum(a, 0) + 0.1
    theta = np.maximum(b, 0) + 0.5
    tgrid = np.arange(0.5, float(LENF), dtype=np.float64)[:, None]
    lg = np.array([math.lgamma(v) for v in aa])
    w = np.exp(-lg) / theta ** aa * tgrid ** (aa - 1.0) * np.exp(-tgrid / theta)
    w = w / w.sum(0)
    UH = (w * (1.0 / M)).astype(f32)  # [LENF, NGRID], mean-over-M folded in
    uh_c = UH.reshape(LENF, NCORES, PPART, CL).transpose(1, 2, 0, 3)

    in_maps = []
    for i in range(NCORES):
        in_maps.append({
            "dd": np.ascontiguousarray(dd_c[i]),
            "pet": np.ascontiguousarray(pet_c[i]),
            "uh": np.ascontiguousarray(uh_c[i]).reshape(PPART, LENF * CL),
        })
    return in_maps


def unpack_outputs(results, T):
    out = np.empty((T, NGRID), np.float32)
    for i in range(NCORES):
        q = results[i]["qr"].reshape(PPART, T, CL)
        out[:, i * NSH : (i + 1) * NSH] = q.transpose(1, 0, 2).reshape(T, NSH)
    return out


_PROG_CACHE = {}


def kernel(x_hydro_model, params_raw, conv_params_hydro):
    from concourse.bass_utils import run_bass_kernel_spmd

    T = x_hydro_model.shape[0]
    key = T
    if key not in _PROG_CACHE:
        _PROG_CACHE[key] = build_program(T=T)
    nc = _PROG_CACHE[key]
    if not nc.is_finalized():
        nc.finalize()
    in_maps = pack_inputs(x_hydro_model, params_raw, conv_params_hydro)
    res = run_bass_kernel_spmd(nc, in_maps, list(range(NCORES)))
    return unpack_outputs(res.results, T)


# revision 17
# speedup vs baseline: 1.8634x; 1.1297x over previous
"""HBV hydrological model (nn_HBVMulTDET_WaterLoss) as a Bass/Tile kernel on
8 Trainium2 NeuronCores.

Strategy: pure data parallelism over the 4000 grid cells (500 cells/core).
Per-core layout: partition p in [0,125) holds 4 cells x 4 components = 16
state lanes in the free dim. All state-free derived quantities are
precomputed on the host, so the device program is a pure steady-state
recurrence stream: the T=365 step loop fully unrolled.

The step time is bound by the serial soil-moisture chain (four DVE->ACT
->DVE round trips ~750ns each plus the dependent DVE links ~210ns each),
so the program is software-pipelined depth 2 around it: step t's emission
carries ONLY the soil chain of step t; the snow chain of step t+1 and the
q-dependent response tail + Q reduce of step t-1 are placed inside step
t's ACT wait windows; the off-critical response section runs on the Pool
engine (whose ISA has no tensor min/max, so all clamps sit on the DVE).
The ACT queue order is lnSM, x1, ln2, x2, then q, so a late response head
can never head-of-line-block the chain activations.

Algebraic restructuring vs the reference:
  - snow melt/refreeze collapsed into one signed flux
        X = max(min(E, SP+SNOW), -MW),  E = melt_cap - refreeze_cap
    meltwater carried negated (NMW); tosoil = NMWn - NMW2 (bit-identical
    to relu(-CWH*SP' - NMW2)); [SPn|NMW2] is ONE 32-lane op (X broadcast)
  - soil pow() via exp/ln: x1 = exp(BETA*ln(SM) - BLF),
    x2 = PET*evap = exp(BETAET*ln(SM1) + lnPET - BETAET*ln(LP*FC))
  - ET/SM update collapsed via SM3 = max(SMc - x2, max(SMc - PET, NZ))
  - capillary in W-form: SM' = SM3*(1 - C*SLZ/FC) + C*SLZ, with the two
    coefficients computed off-path on Pool from the previous SLZ, which
    shortens the on-chain tail from 4 links to 2 (costs ~7e-3 extra
    rounding error from the large-term cancellation, measured 1.41e-2
    total vs the 2e-2 gate)
  - response: rech+exc == SMa-SMc, (1-K) folding with negated states,
    [NSLZn|NSUZn] in ONE 32-lane mult against the adjacent [K2Cn|K1Cn]
    input pair, Q0+Q1+Q2 in one strided-view tensor_reduce per step
All activations resolve into the single natural_log_exp_and_others table
set so the scalar engine never reloads its tables. Gamma unit-hydrograph
weights are computed on host; the routing convolution runs on device.
"""
import math
import numpy as np

T_FULL = 365
NGRID = 4000
NCORES = 8
NSH = NGRID // NCORES      # 500 cells per core
PPART = 125                # partitions used
CL = 4                     # cells per partition
M = 4                      # nmul components
LENF = 15
NZ = 1e-5
TC = 32                    # time-chunk length
NST = 16                   # number of packed per-step streams
WFORM = True               # 2-link capillary tail (False: 4-link exact)

# stream order inside the packed dd tensor; K2Cn/K1Cn are adjacent and
# last so [NSLZn|NSUZn] = [K2Cn|K1Cn] * [SLZ2|SUZ3] is one 32-lane op
DD = ["SNOW", "E", "RAIN", "CWHn", "BETA", "BLF", "FC", "FCinv", "BETAET",
      "LNPB", "C", "PERC", "NUZL", "K0", "K2Cn", "K1Cn"]
DJ = {n: j for j, n in enumerate(DD)}

_TABLES_PATCHED = False


def _patch_act_tables():
    """Strip the functions of natural_log_exp_and_others from every other
    activation table set before the act-table-load CFG pass runs, so all
    activations resolve to that single set and the scalar engine loads its
    tables exactly once."""
    global _TABLES_PATCHED
    if _TABLES_PATCHED:
        return
    import concourse.bacc as bacc
    from concourse import hw_specs

    _orig = hw_specs.get_activation_tables
    target = "natural_log_exp_and_others"

    def _combined_only(arch):
        tables = _orig(arch)
        if target in tables:
            keep = tables[target]
            for name in list(tables):
                if name != target:
                    tables[name] = tables[name] - keep
        return tables

    bacc.get_activation_tables = _combined_only
    _TABLES_PATCHED = True


def build_program(T=T_FULL, tc_len=TC):
    _patch_act_tables()
    import concourse.bass as bass
    import concourse.bacc as bacc
    import concourse.mybir as mybir
    import concourse.tile as tile

    F32 = mybir.dt.float32
    op = mybir.AluOpType
    AF = mybir.ActivationFunctionType

    nc = bacc.Bacc("TRN2")
    dd = nc.declare_dram_parameter("dd", [PPART, T, NST, CL * M], F32,
                                   isOutput=False)
    pet = nc.declare_dram_parameter("pet", [PPART, T, CL], F32, isOutput=False)
    uh = nc.declare_dram_parameter("uh", [PPART, LENF * CL], F32, isOutput=False)
    qr = nc.declare_dram_parameter("qr", [PPART, T, CL], F32, isOutput=True)

    chunks = [(t0, min(tc_len, T - t0)) for t0 in range(0, T, tc_len)]
    n_chunks = len(chunks)

    with tile.TileContext(nc) as tctx:
        with (
            tctx.tile_pool(name="blk", bufs=2) as blk_pool,
            tctx.tile_pool(name="st", bufs=6) as st_pool,
            tctx.tile_pool(name="per", bufs=1) as per_pool,
        ):
            V = nc.vector
            G = nc.gpsimd
            A = nc.scalar
            S = nc.sync

            def tt(eng, out, a, b, o):
                eng.tensor_tensor(out, a, b, o)

            Qfull = per_pool.tile([PPART, (LENF - 1 + T) * CL], F32)
            uh_t = per_pool.tile([PPART, LENF * CL], F32)
            S.dma_start(uh_t[:], uh[:])
            G.memset(Qfull[:, : (LENF - 1) * CL], 0.0)

            # ---- state bootstrap ----
            SM = st_pool.tile([PPART, 16], F32, tag="SM")
            G.memset(SM[:], 0.001)
            TM_prev = st_pool.tile([PPART, 32], F32, tag="TM")
            G.memset(TM_prev[:, 0:16], 0.001)      # SP0
            TSP_cur = st_pool.tile([PPART, 32], F32, tag="TSP")
            G.memset(TSP_cur[:, 16:32], -0.001)    # NMW0
            # comb: 8 slots of 16 lanes; lane = g*32 + x*16:
            #  g0x0 SUZ2 | g1x0 SLZ2, g1x1 SUZ3 | g2x0 NSLZn | g3x0 NSUZn
            pc = st_pool.tile([PPART, 128], F32, tag="comb")
            G.memset(pc[:, 64:80], -0.001)    # NSLZ
            G.memset(pc[:, 96:112], -0.001)   # NSUZ

            def nt(tag):
                return st_pool.tile([PPART, 16], F32, tag=tag, name=tag)

            def emit_dma(ci):
                t0, tcn = chunks[ci]
                dt_ = blk_pool.tile([PPART, tc_len * NST * 16], F32,
                                    tag="dd", name=f"dd_{t0}")
                S.dma_start(
                    dt_[:, : tcn * NST * 16].rearrange(
                        "p (t j f) -> p t j f", j=NST, f=16),
                    dd[:, t0 : t0 + tcn, :, :],
                )
                pt = blk_pool.tile([PPART, tc_len * CL], F32, tag="PET",
                                   name=f"PET_{t0}")
                S.dma_start(
                    pt[:, : tcn * CL].rearrange("p (t c) -> p t c", c=CL),
                    pet[:, t0 : t0 + tcn, :],
                )
                petb = (
                    pt[:, : tcn * CL]
                    .rearrange("p (t c) -> p t c", c=CL)
                    .unsqueeze(3)
                    .to_broadcast((PPART, tcn, CL, M))
                )
                return {"t0": t0, "tcn": tcn, "dt": dt_, "PETb": petb}

            bufs = [emit_dma(0)]

            def cs_at(t, name):
                ci = t // tc_len
                b = bufs[ci]
                ti = t - b["t0"]
                base = ti * NST * 16 + DJ[name] * 16
                return b["dt"][:, base : base + 16]

            # ---- snow sub-chain (DVE), software-pipelined one step ahead:
            # results land in sn dicts keyed by step.
            def emit_snow_a(t):
                """SPa, mn, X, [SPn|NMW2] for step t."""
                nonlocal TSP_cur
                tt(G, TSP_cur[:, 0:16], TM_prev[:, 0:16], cs_at(t, "SNOW"),
                   op.add)                         # SPa
                mn = nt("mn")
                tt(V, mn[:], cs_at(t, "E"), TSP_cur[:, 0:16], op.min)
                X = nt("X")
                tt(V, X[:], mn[:], TSP_cur[:, 16:32], op.max)
                TM = st_pool.tile([PPART, 32], F32, tag="TMn", name="TM")
                tt(V,
                   TM[:].rearrange("p (g f) -> p g f", g=2),
                   TSP_cur[:].rearrange("p (g f) -> p g f", g=2),
                   X[:].unsqueeze(1).to_broadcast((PPART, 2, 16)),
                   op.subtract)                    # [SPn | NMW2]
                return TM

            def emit_snow_b(t, TM):
                """NW, NMWn, tosp, wi for step t; advances TM_prev/TSP_cur."""
                nonlocal TM_prev, TSP_cur
                NW = nt("NW")
                tt(V, NW[:], cs_at(t, "CWHn"), TM[:, 0:16], op.mult)
                TSP_next = st_pool.tile([PPART, 32], F32, tag="TSP",
                                        name="TSP")
                tt(V, TSP_next[:, 16:32], TM[:, 16:32], NW[:], op.max)  # NMWn
                tosp = nt("tosp")
                tt(V, tosp[:], TSP_next[:, 16:32], TM[:, 16:32], op.subtract)
                wi = nt("wi")
                tt(G, wi[:], cs_at(t, "RAIN"), tosp[:], op.add)
                TM_prev = TM
                TSP_cur = TSP_next
                return wi

            pendR = None
            pendQ = None

            def emit_pendR(p):
                """q-dependent response tail of the previous step (Pool)."""
                if p is None:
                    return
                cb = p["comb"]
                Q0 = nt("Q0")
                tt(G, Q0[:], p["K0"], p["q"][:], op.mult)
                tt(G, cb[:, 48:64], cb[:, 0:16], Q0[:], op.subtract)  # SUZ3
                # [NSLZn|NSUZn] = [K2Cn|K1Cn] * [SLZ2|SUZ3]  (one op)
                tt(G,
                   cb[:, 64:128].rearrange("p (g f) -> p g f", g=2)[:, :, 0:16],
                   p["K1K2"],
                   cb[:, 32:64].rearrange("p (g f) -> p g f", g=2),
                   op.mult)

            def emit_pendQ(p):
                if p is None:
                    return
                V.tensor_reduce(
                    Qfull[:, (LENF - 1 + p["t"]) * CL : (LENF + p["t"]) * CL],
                    p["comb"][:].rearrange("p (g x c m) -> p x c g m",
                                           g=4, x=2, m=M)[:, 0],
                    axis=mybir.AxisListType.XY,
                    op=op.add,
                )

            # ---- prologue: snow for step 0 ----
            TM0 = emit_snow_a(0)
            wi = emit_snow_b(0, TM0)

            for t in range(T):
                ci = t // tc_len
                # prefetch the next chunk a full chunk ahead
                if t % tc_len == 0 and ci + 1 < n_chunks and len(bufs) == ci + 1:
                    bufs.append(emit_dma(ci + 1))

                def cs(name):
                    return cs_at(t, name)

                # ---- chain head ----
                lnSM = nt("lnSM")
                A.activation(lnSM[:], SM[:], AF.Ln)

                # lnSM window: prev response tail (Pool) + W-prep (Pool)
                emit_pendR(pendR)
                NSLZ = pc[:, 64:80]
                NSUZ = pc[:, 96:112]
                CnSLZ = nt("CnSLZ")
                tt(G, CnSLZ[:], cs("C"), NSLZ, op.mult)
                if WFORM:
                    CF = nt("CF")
                    tt(G, CF[:], CnSLZ[:], cs("FCinv"), op.mult)

                SMa = nt("SMa")
                tt(G, SMa[:], SM[:], wi[:], op.add)
                v = nt("v")
                tt(V, v[:], lnSM[:], cs("BETA"), op.mult)
                u = nt("u")
                tt(V, u[:], v[:], cs("BLF"), op.subtract)
                x1 = nt("x1")
                A.activation(x1[:], u[:], AF.Exp)

                # x1 window: next step's snow head + prev step's Q reduce
                TMn = emit_snow_a(t + 1) if t + 1 < T else None
                emit_pendQ(pendQ)

                rech = nt("rech")
                V.scalar_tensor_tensor(rech[:], x1[:], 1.0, wi[:],
                                       op.min, op.mult)
                SM1 = nt("SM1")
                tt(V, SM1[:], SMa[:], rech[:], op.subtract)
                ln2 = nt("ln2")
                A.activation(ln2[:], SM1[:], AF.Ln)

                # ln2 window: next step's snow tail + SMc + response head
                wi_next = emit_snow_b(t + 1, TMn) if t + 1 < T else None
                SMc = nt("SMc")
                tt(V, SMc[:], SM1[:], cs("FC"), op.min)
                SMcP = nt("SMcP")
                tt(G, SMcP[:].rearrange("p (c m) -> p c m", m=M),
                   SMc[:].rearrange("p (c m) -> p c m", m=M),
                   bufs[ci]["PETb"][:, t - bufs[ci]["t0"], :, :], op.subtract)
                SMcP2 = nt("SMcP2")
                V.tensor_scalar_max(SMcP2[:], SMcP[:], NZ)
                SUZ1a = nt("SUZ1a")
                tt(G, SUZ1a[:], SMa[:], NSUZ, op.subtract)
                SUZ1 = nt("SUZ1")
                tt(G, SUZ1[:], SUZ1a[:], SMc[:], op.subtract)
                PERCa = nt("PERCa")
                tt(V, PERCa[:], SUZ1[:], cs("PERC"), op.min)
                comb = st_pool.tile([PPART, 128], F32, tag="comb",
                                    name="comb")
                tt(G, comb[:, 0:16], SUZ1[:], PERCa[:], op.subtract)  # SUZ2
                t5 = nt("t5")
                tt(G, t5[:], comb[:, 0:16], cs("NUZL"), op.add)

                v2 = nt("v2")
                tt(V, v2[:], ln2[:], cs("BETAET"), op.mult)
                w2 = nt("w2")
                tt(V, w2[:], v2[:], cs("LNPB"), op.add)
                x2 = nt("x2")
                A.activation(x2[:], w2[:], AF.Exp)
                q = nt("q")
                A.activation(q[:], t5[:], AF.Relu)
                if WFORM:
                    Wc = nt("Wc")
                    A.add(Wc[:], CF[:], 1.0)

                # ---- chain tail ----
                tq = nt("tq")
                V.scalar_tensor_tensor(tq[:], x2[:], -1.0, SMc[:],
                                       op.mult, op.add)
                SM3 = nt("SM3")
                tt(V, SM3[:], tq[:], SMcP2[:], op.max)
                if WFORM:
                    pp = nt("pp")
                    tt(V, pp[:], SM3[:], Wc[:], op.mult)
                    SMn = nt("SM")
                    tt(V, SMn[:], pp[:], CnSLZ[:], op.subtract)
                else:
                    g_ = nt("g")
                    tt(V, g_[:], SM3[:], cs("FCinv"), op.mult)
                    rln = nt("rln")
                    V.tensor_scalar(rln[:], g_[:], 1.0, 1.0, op.min,
                                    op.subtract)
                    capv = nt("capv")
                    tt(V, capv[:], CnSLZ[:], rln[:], op.mult)
                    SMn = nt("SM")
                    tt(V, SMn[:], SM3[:], capv[:], op.add)
                SM = SMn

                # ---- response tail (off-chain) ----
                cap = nt("cap")
                tt(G, cap[:], SMn[:], SM3[:], op.subtract)
                sl_n = nt("sl_n")
                tt(G, sl_n[:], NSLZ, cap[:], op.add)
                NSLZ1 = nt("NSLZ1")
                V.tensor_scalar_min(NSLZ1[:], sl_n[:], -NZ)
                tt(G, comb[:, 32:48], PERCa[:], NSLZ1[:], op.subtract)  # SLZ2

                kkbase = (t - bufs[ci]["t0"]) * NST * 16 + DJ["K2Cn"] * 16
                K1K2 = bufs[ci]["dt"][:, kkbase : kkbase + 32].rearrange(
                    "p (g f) -> p g f", g=2)
                pendR = {"comb": comb, "q": q, "K0": cs("K0"), "K1K2": K1K2}
                pendQ = {"t": t, "comb": comb}
                pc = comb
                wi = wi_next

            emit_pendR(pendR)
            emit_pendQ(pendQ)

            # ---- gamma-UH routing (DVE, bulk) ----
            Qr = per_pool.tile([PPART, T * CL], F32)
            prod = per_pool.tile([PPART, T * CL], F32)

            def qr4(ap_):
                return ap_.rearrange("p (t c) -> p t c", c=CL)

            for k in range(LENF):
                sh = Qfull[:, (LENF - 1 - k) * CL : (LENF - 1 - k + T) * CL]
                uhk = (
                    uh_t[:, k * CL : (k + 1) * CL]
                    .unsqueeze(1)
                    .to_broadcast((PPART, T, CL))
                )
                if k == 0:
                    tt(V, qr4(Qr[:]), uhk, qr4(sh), op.mult)
                else:
                    tt(V, qr4(prod[:]), uhk, qr4(sh), op.mult)
                    tt(V, qr4(Qr[:]), qr4(Qr[:]), qr4(prod[:]), op.add)

            S.dma_start(qr[:, :, :], Qr[:].rearrange("p (t c) -> p t c", c=CL))

    return nc


# ---------------- host-side packing ----------------

def _derived_full(x_hydro_model, params_raw):
    """All state-free per-step tensors, float32, shapes [T, N, M] (per-cell
    quantities broadcast over M)."""
    f32 = np.float32
    T, N, _ = x_hydro_model.shape
    raw = np.ascontiguousarray(params_raw[:, :, :14, :], dtype=f32)
    x = np.ascontiguousarray(x_hydro_model, dtype=f32)
    P = x[:, :, 0:1]
    Ta = x[:, :, 1:2]
    PET = x[:, :, 2:3]

    BETA = f32(5.0) * raw[:, :, 0] + f32(1.0)
    FC = f32(950.0) * raw[:, :, 1] + f32(50.0)
    K0 = f32(0.85) * raw[:, :, 2] + f32(0.05)
    K1Cn = f32(0.49) * raw[:, :, 3] - f32(0.99)
    K2Cn = f32(0.199) * raw[:, :, 4] - f32(0.999)
    LP = f32(0.8) * raw[:, :, 5] + f32(0.2)
    PERC = f32(10.0) * raw[:, :, 6]
    NUZL = f32(-100.0) * raw[:, :, 7]
    TTn = f32(-5.0) * raw[:, :, 8] + f32(2.5)
    CFMX = f32(9.5) * raw[:, :, 9] + f32(0.5)
    CWHn = f32(-0.2) * raw[:, :, 11]
    BETAET = f32(4.7) * raw[:, :, 12] + f32(0.3)
    C = raw[:, :, 13]

    Tdiff = (Ta + TTn).astype(f32)
    m1 = (CFMX * Tdiff).astype(f32)
    rn = np.maximum(-m1, 0).astype(f32)
    Rc0 = ((f32(0.1) * raw[:, :, 10]).astype(f32) * rn).astype(f32)
    Gc0 = np.maximum(m1, 0).astype(f32)
    E = (Gc0 - Rc0).astype(f32)
    mask = (Tdiff >= 0).astype(f32)
    RAIN = (mask * P).astype(f32)
    SNOW = (P - RAIN).astype(f32)
    lnFC = np.log(FC).astype(f32)
    FCinv = np.exp(-lnFC).astype(f32)
    BLF = (BETA * lnFC).astype(f32)
    LPFC = (LP * FC).astype(f32)
    lnLPFC = np.log(LPFC).astype(f32)
    BL2 = (BETAET * lnLPFC).astype(f32)
    lnPET = np.log(np.maximum(PET, f32(1e-30))).astype(f32)
    LNPB = (lnPET - BL2).astype(f32)

    return {
        "E": E, "SNOW": SNOW, "RAIN": RAIN, "CWHn": CWHn, "BETA": BETA,
        "BLF": BLF, "FC": FC, "FCinv": FCinv, "BETAET": BETAET, "LNPB": LNPB,
        "C": C, "PERC": PERC, "NUZL": NUZL, "K0": K0, "K1Cn": K1Cn,
        "K2Cn": K2Cn,
    }


def pack_inputs(x_hydro_model, params_raw, conv_params_hydro):
    T = x_hydro_model.shape[0]
    f32 = np.float32
    der = _derived_full(x_hydro_model, params_raw)
    # [T, N, M] -> per core [PPART, T, NST, CL*M]
    dd_full = np.stack([der[n] for n in DD], axis=0)  # [nd, T, N, M]
    nd = dd_full.shape[0]
    dd_c = (dd_full.reshape(nd, T, NCORES, PPART, CL * M)
            .transpose(2, 3, 1, 0, 4))           # [cores, P, T, nd, 16]

    PET = np.ascontiguousarray(x_hydro_model[:, :, 2], dtype=f32)  # [T, N]
    pet_c = PET.reshape(T, NCORES, PPART, CL).transpose(1, 2, 0, 3)

    conv = np.asarray(conv_params_hydro, dtype=np.float64)
    a = conv[:, 0] * 2.9
    b = conv[:, 1] * 6.5
    aa = np.maximum(a, 0) + 0.1
    theta = np.maximum(b, 0) + 0.5
    tgrid = np.arange(0.5, float(LENF), dtype=np.float64)[:, None]
    lg = np.array([math.lgamma(v) for v in aa])
    w = np.exp(-lg) / theta ** aa * tgrid ** (aa - 1.0) * np.exp(-tgrid / theta)
    w = w / w.sum(0)
    UH = (w * (1.0 / M)).astype(f32)  # [LENF, NGRID], mean-over-M folded in
    uh_c = UH.reshape(LENF, NCORES, PPART, CL).transpose(1, 2, 0, 3)

    in_maps = []
    for i in range(NCORES):
        in_maps.append({
            "dd": np.ascontiguousarray(dd_c[i]),
            "pet": np.ascontiguousarray(pet_c[i]),
            "uh": np.ascontiguousarray(uh_c[i]).reshape(PPART, LENF * CL),
        })
    return in_maps


def unpack_outputs(results, T):
    out = np.empty((T, NGRID), np.float32)
    for i in range(NCORES):
        q = results[i]["qr"].reshape(PPART, T, CL)
        out[:, i * NSH : (i + 1) * NSH] = q.transpose(1, 0, 2).reshape(T, NSH)
    return out


_PROG_CACHE = {}


def kernel(x_hydro_model, params_raw, conv_params_hydro):
    from concourse.bass_utils import run_bass_kernel_spmd

    T = x_hydro_model.shape[0]
    key = T
    if key not in _PROG_CACHE:
        _PROG_CACHE[key] = build_program(T=T)
    nc = _PROG_CACHE[key]
    if not nc.is_finalized():
        nc.finalize()
    in_maps = pack_inputs(x_hydro_model, params_raw, conv_params_hydro)
    res = run_bass_kernel_spmd(nc, in_maps, list(range(NCORES)))
    return unpack_outputs(res.results, T)
